# revision 1
# baseline (speedup 1.0000x reference)
"""Grouped-correlation cost volume (CostVolume) Bass kernel for Trainium2.

Problem: x, y: (4, 512, 128, 256) f32; GROUP=4, MAXDISP=48, D=49.
out[b, g, k, h, w] = sum_cg x[b, 128g+cg, h, w] * y[b, 128g+cg, h, w-k]
(zero where w < k), out shape (4, 4, 49, 128, 256).

Strategy: shard the 16 (b, g) units over 8 cores (2 each; the channel sum is
within-group, so no cross-core reduce). Per (unit, h) row the correlation is
a banded Gram matrix between x columns and y columns with contraction over
cg = 128 = the TensorE partition dim. To keep the stored band nearly
rectangular, each 128-wide w-block is split into four M=32 column groups
(tile_position col-tiling) whose y-windows are shifted by the group base:

  P[32m+i', 80t+j'] = sum_cg x[cg, 128t+32m+i'] * ypad[cg, 128t+32m-48+j']

so the useful entries are j' = i' + 48 - k with i' in [0,32), j' in [0,80) —
a 32x80 parallelogram per group (1.63x amplification instead of 3.6x for
M=128). The per-(unit,h) (128, 160) PSUM tile is copied to SBUF and stored
to DRAM as-is; the band extraction (a pure gather) happens on the host
during the unshard step.

The module is built through bacc (not raw bass) so excess semaphore waits
get split onto EventSemaphore instructions — TRN2 allows at most one sync
wait per regular instruction.
"""

import os

import numpy as np

import concourse.bass as bass
import concourse.mybir as mybir
import concourse.tile as tile
from concourse import bacc

MAXDISP = 48
D = MAXDISP + 1          # 49 disparities
CG = 128                 # channels per group = contraction dim
GROUP = 4
B = 4
H = 128
W = 256
NB = W // 128            # 2 w-blocks of 128
NM = 4                   # M=32 col groups per w-block
MW = 32                  # group width
NWIN = MAXDISP + MW      # 80: y window per group
N_CORES = 8
N_UNITS = 2              # (b,g) units per core
ROW = 384                # padded y row: 48 left pad + 256 data + 80 slack
NY_BUFS = 3

_last_results = None     # BassKernelResults of the most recent run (for test.py)


def build_nc(n_units=N_UNITS, n_h=H, h_chunk=16):
    """Build the per-core Bass module (fp32)."""
    assert n_h % h_chunk == 0
    n_chunks = n_h // h_chunk
    f32 = mybir.dt.float32
    rect_w = NB * NWIN   # 160 columns per (unit, h)

    nc = bacc.Bacc()
    x = nc.dram_tensor("x", [n_units, CG, n_h, W], f32, kind="ExternalInput")
    y = nc.dram_tensor("y", [n_units, CG, n_h, W], f32, kind="ExternalInput")
    out = nc.dram_tensor(
        "out", [n_units, n_chunks, 128, h_chunk, rect_w], f32, kind="ExternalOutput"
    )

    y_flat_len = ROW * h_chunk + 48  # slack so the pad memset can cover tails

    with tile.TileContext(nc) as tc:
        with (
            tc.tile_pool(name="io", bufs=2) as io_pool,
            tc.tile_pool(name="ybufs", bufs=1) as y_pool,
            tc.tile_pool(name="work", bufs=2) as work_pool,
            tc.tile_pool(name="psum_mm", bufs=4, space="PSUM") as psum_mm,
        ):
            # persistent y buffers with pads zeroed once (DVE so the pad
            # dependency rides the DVE semaphore, not Pool)
            y_bufs = []
            for i in range(NY_BUFS):
                yb = y_pool.tile([128, y_flat_len], f32, name=f"y_buf{i}")
                nc.vector.memset(yb[:, 0:48], 0.0)
                pad2 = bass.AP(
                    tensor=yb.tensor,
                    offset=yb.offset + 304,
                    ap=[[y_flat_len, 128], [ROW, h_chunk], [1, 128]],
                )
                nc.vector.memset(pad2, 0.0)
                y_bufs.append(yb)

            for u in range(n_units):
                for hc in range(n_chunks):
                    h0 = hc * h_chunk
                    x_tile = io_pool.tile([128, h_chunk, W], f32, name="x_tile", tag="x")
                    nc.sync.dma_start(out=x_tile, in_=x[u, :, h0 : h0 + h_chunk, :])

                    y_tile = y_bufs[(u * n_chunks + hc) % NY_BUFS]
                    # y rows land at [h*ROW + 48, h*ROW + 304)
                    y_dst = bass.AP(
                        tensor=y_tile.tensor,
                        offset=y_tile.offset + 48,
                        ap=[[y_flat_len, 128], [ROW, h_chunk], [1, W]],
                    )
                    nc.scalar.dma_start(out=y_dst, in_=y[u, :, h0 : h0 + h_chunk, :])

                    # per-chunk staging tile so the store is one big DMA
                    s_big = work_pool.tile(
                        [128, h_chunk, rect_w], f32, name="s_big", tag="S"
                    )
                    for h in range(h_chunk):
                        p_mm = psum_mm.tile([128, rect_w], f32, name="p_mm", tag="P")
                        for t in range(NB):
                            for m in range(NM):
                                base = 128 * t + MW * m
                                lhsT = x_tile[:, h, base : base + MW]
                                # tile coords: ypad[w2] at 48 + w2
                                rhs = y_tile[:, h * ROW + base : h * ROW + base + NWIN]
                                nc.tensor.matmul(
                                    p_mm[MW * m : MW * (m + 1),
                                         NWIN * t : NWIN * (t + 1)],
                                    lhsT,
                                    rhs,
                                    start=True,
                                    stop=True,
                                    tile_position=(0, MW * m),
                                )
                        nc.vector.tensor_copy(s_big[:, h, :], p_mm)
                    # chunk-major out layout: 10 KB contiguous per partition
                    st_eng = nc.sync if (hc % 2 == 0) else nc.scalar
                    st_eng.dma_start(out=out[u, hc], in_=s_big)

    nc.finalize()
    return nc


def _shard_inputs(x, y):
    """x, y: (4, 512, 128, 256) -> per-core dicts of (2, 128, 128, 256)."""
    xu = x.reshape(B * GROUP, CG, H, W)
    yu = y.reshape(B * GROUP, CG, H, W)
    in_maps = []
    for c in range(N_CORES):
        in_maps.append(
            {
                "x": np.ascontiguousarray(xu[2 * c : 2 * c + 2]),
                "y": np.ascontiguousarray(yu[2 * c : 2 * c + 2]),
            }
        )
    return in_maps


def _extract_band(rect, n_h=H):
    """rect: (n, n_chunks, 128, h_chunk, 160) rects -> (n, D, n_h, W) volume.

    rect[n, hc, 32m+i, h', 80t+j] = out[n, 48-(j-i), hc*h_chunk+h', 128t+32m+i]
    for j-i in [0, 48].
    """
    n, n_chunks, _, h_chunk, _ = rect.shape
    r = rect.reshape(n, n_chunks, NM, MW, h_chunk, NB, NWIN)  # [n,hc,m,i,h',t,j]
    idx = np.arange(MW)[:, None] + np.arange(D)[None, :]      # j = i + c
    g = np.take_along_axis(
        r, idx[None, None, None, :, None, None, :], axis=-1
    )  # [n, hc, m, i, h', t, c]
    g = g.transpose(0, 6, 1, 4, 5, 2, 3)                      # [n,c,hc,h',t,m,i]
    g = g.reshape(n, D, n_h, W)[:, ::-1]                      # c -> k = 48 - c
    return np.ascontiguousarray(g)


def kernel(x, y):
    global _last_results
    from concourse.bass_utils import run_bass_kernel_spmd

    x = np.ascontiguousarray(np.asarray(x), dtype=np.float32)
    y = np.ascontiguousarray(np.asarray(y), dtype=np.float32)

    nc = build_nc()
    in_maps = _shard_inputs(x, y)
    trace = bool(int(os.environ.get("COSTVOL_TRACE", "0")))
    results = run_bass_kernel_spmd(
        nc,
        in_maps,
        core_ids=list(range(N_CORES)),
        trace=trace,
    )
    _last_results = results

    rects = np.concatenate([r["out"] for r in results.results], axis=0)
    full = _extract_band(rects)  # (16, D, H, W)
    return full.reshape(B, GROUP, D, H, W)



# revision 2
# speedup vs baseline: 2.1327x; 2.1327x over previous
"""Grouped-correlation cost volume (CostVolume) Bass kernel for Trainium2.

Problem: x, y: (4, 512, 128, 256) f32; GROUP=4, MAXDISP=48, D=49.
out[b, g, k, h, w] = sum_cg x[b, 128g+cg, h, w] * y[b, 128g+cg, h, w-k]
(zero where w < k), out shape (4, 4, 49, 128, 256).

Strategy: shard the 16 (b, g) units over 8 cores (2 each; the channel sum is
within-group, so no cross-core reduce). Per (unit, h) row the correlation is
a banded Gram matrix between x columns and y columns with contraction over
cg = 128 = the TensorE partition dim. Each 128-wide w-block is split into
four M=32 column groups (tile_position col-tiling) whose y-windows are
shifted by the group base:

  P[32m+i', 80t+j'] = sum_cg x[cg, 128t+32m+i'] * y[cg, 128t+32m-48+j']

so the useful entries are j' = i' + 48 - k with i' in [0,32), j' in [0,80) —
a 32x80 parallelogram per group (1.63x amplification instead of 3.6x for
M=128). The per-(unit,h) (128, 160) PSUM tile is copied to SBUF and stored
to DRAM as-is; the band extraction (a pure gather) happens on the host
during the unshard step.

The whole pipeline runs in bf16 (inputs cast on host, matmul at 1 cyc/row
vs fp32's 4, output staged bf16) — the rel-err budget is 2e-2 and bf16
contributes ~5e-3. y is loaded contiguously (no padded rows): windows that
reach before y col 0 read garbage (prior row tail / uninit SBUF), which
only lands in the w < k entries of the volume; the host zeroes those after
the band gather, matching the reference's zero padding exactly.

The module is built through bacc (not raw bass) so excess semaphore waits
get split onto EventSemaphore instructions — TRN2 allows at most one sync
wait per regular instruction.
"""

import os

import numpy as np
import ml_dtypes

import concourse.bass as bass
import concourse.mybir as mybir
import concourse.tile as tile
from concourse import bacc

MAXDISP = 48
D = MAXDISP + 1          # 49 disparities
CG = 128                 # channels per group = contraction dim
GROUP = 4
B = 4
H = 128
W = 256
NB = W // 128            # 2 w-blocks of 128
NM = 4                   # M=32 col groups per w-block
MW = 32                  # group width
NWIN = MAXDISP + MW      # 80: y window per group
N_CORES = 8
N_UNITS = 2              # (b,g) units per core

_last_results = None     # BassKernelResults of the most recent run (for test.py)


def build_nc(n_units=N_UNITS, n_h=H, h_chunk=16):
    """Build the per-core Bass module (bf16)."""
    assert n_h % h_chunk == 0
    n_chunks = n_h // h_chunk
    bf16 = mybir.dt.bfloat16
    f32 = mybir.dt.float32
    rect_w = NB * NWIN   # 160 columns per (unit, h)
    y_len = MAXDISP + h_chunk * W   # 48-col garbage prefix + contiguous rows

    nc = bacc.Bacc()
    x = nc.dram_tensor("x", [n_units, CG, n_h, W], bf16, kind="ExternalInput")
    y = nc.dram_tensor("y", [n_units, CG, n_h, W], bf16, kind="ExternalInput")
    out = nc.dram_tensor(
        "out", [n_units, n_chunks, 128, h_chunk, rect_w], bf16, kind="ExternalOutput"
    )

    with tile.TileContext(nc) as tc:
        with (
            tc.tile_pool(name="io", bufs=2) as io_pool,
            tc.tile_pool(name="ybufs", bufs=2) as y_pool,
            tc.tile_pool(name="work", bufs=2) as work_pool,
            tc.tile_pool(name="psum_mm", bufs=6, space="PSUM") as psum_mm,
        ):
            for u in range(n_units):
                for hc in range(n_chunks):
                    h0 = hc * h_chunk
                    x_tile = io_pool.tile([128, h_chunk, W], bf16, name="x_tile", tag="x")
                    nc.sync.dma_start(out=x_tile, in_=x[u, :, h0 : h0 + h_chunk, :])

                    y_tile = y_pool.tile([128, y_len], bf16, name="y_tile", tag="y")
                    # y rows land contiguously at [48, 48 + h_chunk*W)
                    y_dst = bass.AP(
                        tensor=y_tile.tensor,
                        offset=y_tile.offset + MAXDISP,
                        ap=[[y_len, 128], [W, h_chunk], [1, W]],
                    )
                    nc.sync.dma_start(out=y_dst, in_=y[u, :, h0 : h0 + h_chunk, :])

                    # per-chunk staging tile so the store is one big DMA
                    s_big = work_pool.tile(
                        [128, h_chunk, rect_w], bf16, name="s_big", tag="S"
                    )
                    for h in range(h_chunk):
                        p_mm = psum_mm.tile([128, rect_w], f32, name="p_mm", tag="P")
                        for t in range(NB):
                            for m in range(NM):
                                base = 128 * t + MW * m
                                lhsT = x_tile[:, h, base : base + MW]
                                # window = y cols [base-48, base+32) at tile
                                # cols [h*W + base, +80)
                                rhs = y_tile[:, h * W + base : h * W + base + NWIN]
                                nc.tensor.matmul(
                                    p_mm[MW * m : MW * (m + 1),
                                         NWIN * t : NWIN * (t + 1)],
                                    lhsT,
                                    rhs,
                                    start=True,
                                    stop=True,
                                    tile_position=(0, MW * m),
                                )
                        # alternate copy engines so neither DVE nor Act
                        # saturates
                        if h % 2 == 0:
                            nc.vector.tensor_copy(s_big[:, h, :], p_mm)
                        else:
                            nc.scalar.copy(s_big[:, h, :], p_mm)
                    # chunk-major out layout: 5 KB contiguous per partition
                    nc.gpsimd.dma_start(out=out[u, hc], in_=s_big)

    nc.finalize()
    return nc


def _shard_inputs(x, y):
    """x, y: (4, 512, 128, 256) bf16 -> per-core dicts of (2, 128, 128, 256)."""
    xu = x.reshape(B * GROUP, CG, H, W)
    yu = y.reshape(B * GROUP, CG, H, W)
    in_maps = []
    for c in range(N_CORES):
        in_maps.append(
            {
                "x": np.ascontiguousarray(xu[2 * c : 2 * c + 2]),
                "y": np.ascontiguousarray(yu[2 * c : 2 * c + 2]),
            }
        )
    return in_maps


def _extract_band(rect, n_h=H):
    """rect: (n, n_chunks, 128, h_chunk, 160) rects -> (n, D, n_h, W) volume.

    rect[n, hc, 32m+i, h', 80t+j] = out[n, 48-(j-i), hc*h_chunk+h', 128t+32m+i]
    for j-i in [0, 48].
    """
    n, n_chunks, _, h_chunk, _ = rect.shape
    r = rect.reshape(n, n_chunks, NM, MW, h_chunk, NB, NWIN)  # [n,hc,m,i,h',t,j]
    idx = np.arange(MW)[:, None] + np.arange(D)[None, :]      # j = i + c
    g = np.take_along_axis(
        r, idx[None, None, None, :, None, None, :], axis=-1
    )  # [n, hc, m, i, h', t, c]
    g = g.transpose(0, 6, 1, 4, 5, 2, 3)                      # [n,c,hc,h',t,m,i]
    g = g.reshape(n, D, n_h, W)[:, ::-1]                      # c -> k = 48 - c
    g = np.ascontiguousarray(g).astype(np.float32)
    # zero the out-of-range band (w < k): the kernel leaves garbage there
    for k in range(1, D):
        g[:, k, :, :k] = 0.0
    return g


def kernel(x, y):
    global _last_results
    from concourse.bass_utils import run_bass_kernel_spmd

    x = np.asarray(x, dtype=np.float32).astype(ml_dtypes.bfloat16)
    y = np.asarray(y, dtype=np.float32).astype(ml_dtypes.bfloat16)

    nc = build_nc()
    in_maps = _shard_inputs(x, y)
    trace = bool(int(os.environ.get("COSTVOL_TRACE", "0")))
    results = run_bass_kernel_spmd(
        nc,
        in_maps,
        core_ids=list(range(N_CORES)),
        trace=trace,
    )
    _last_results = results

    rects = np.concatenate([r["out"] for r in results.results], axis=0)
    full = _extract_band(rects)  # (16, D, H, W)
    return full.reshape(B, GROUP, D, H, W)
